# revision 1
# baseline (speedup 1.0000x reference)
"""Trainium2 Bass kernel for nn_BPF_Attention (B=4, N=2048, C=768, H=12).

Sharding: 8 cores = 4 batches x 2 head-groups (6 heads each).
Each core computes, for its (batch b, head-group g):
  qT/kT = (w_qkv_g^T x_b^T) directly in [d, n] layout (no on-device transpose)
  v     = x_b @ w_v_g in natural [n, d] layout
  S^T[k,q] = kT^T-chunks x qT  (PE, bf16)
  P^T = exp(0.125*S^T) * m01T  (ACT exp from PSUM; DVE mask multiply)
  outT[d,q] (+ denom row via ones-column in lhsT) = v_ones^T-chunks x P^T
  attn = outT * (1/denom)  (DVE; gpsimd partition-broadcast of recip row)
  y_partial = attn^T-chunks @ w_proj_g  (PE) -> DMA from PSUM to DRAM
Host sums the two partials per batch and adds b_proj.
"""

import numpy as np
import ml_dtypes

import concourse.bass as bass
import concourse.tile as tile
import concourse.bacc as bacc
import concourse.mybir as mybir
from concourse.bass_utils import run_bass_kernel_spmd

BF16 = ml_dtypes.bfloat16
F32 = mybir.dt.float32
BF = mybir.dt.bfloat16

B, N, C = 4, 2048, 768
H, D = 12, 64
HG = 6                      # heads per core
SCALE = D ** -0.5           # 0.125
NCORES = 8
WQ = 512                    # query-chunk width in attention inner loop
NKT = N // 128              # 16 k-tiles
NQC = N // WQ               # 2 q-chunks


DEBUG_DUMPS = False
ABLATE = set()


def _build_nc(loop=0):
    nc = bacc.Bacc(
        "TRN2",
        target_bir_lowering=False,
        debug=False,
        enable_asserts=True,
        num_devices=NCORES,
    )
    xT_d = nc.dram_tensor("xT", [6, 128, N], BF, kind="ExternalInput")
    w_d = nc.dram_tensor("wqkv", [6, 128, 1152], BF, kind="ExternalInput")
    m_d = nc.dram_tensor("maskT", [NKT, 128, N], BF, kind="ExternalInput")
    wp_d = nc.dram_tensor("wp", [3, 128, C], BF, kind="ExternalInput")
    y_d = nc.dram_tensor("y", [N, C], F32, kind="ExternalOutput")
    dbg = None
    if DEBUG_DUMPS:
        dbg = {
            "d_qT": nc.dram_tensor("d_qT", [128, 3, N], BF, kind="ExternalOutput"),
            "d_kT": nc.dram_tensor("d_kT", [128, 3, N], BF, kind="ExternalOutput"),
            "d_vo": nc.dram_tensor("d_vo", [128, NKT, HG, 65], BF, kind="ExternalOutput"),
            "d_attn": nc.dram_tensor("d_attn", [128, 3, N], BF, kind="ExternalOutput"),
            "d_praw": nc.dram_tensor("d_praw", [128, WQ], BF, kind="ExternalOutput"),
            "d_pt": nc.dram_tensor("d_pt", [128, WQ], BF, kind="ExternalOutput"),
            "d_ot": nc.dram_tensor("d_ot", [128, WQ], F32, kind="ExternalOutput"),
        }

    with tile.TileContext(nc) as tc:
        if loop:
            with tc.For_i(0, loop, 1):
                _kernel_body(tc, xT_d, w_d, m_d, wp_d, y_d, dbg)
        else:
            _kernel_body(tc, xT_d, w_d, m_d, wp_d, y_d, dbg)
    nc.compile()
    return nc


def _kernel_body(tc, xT_d, w_d, m_d, wp_d, y_d, dbg=None):
    nc = tc.nc
    from contextlib import ExitStack

    with ExitStack() as ctx:
        persist = ctx.enter_context(tc.tile_pool(name="persist", bufs=1))
        work = ctx.enter_context(tc.tile_pool(name="work", bufs=3))
        work2 = ctx.enter_context(tc.tile_pool(name="work2", bufs=2))

        # ---- persistent SBUF tensors ----
        xT_sb = persist.tile([128, 6, N], BF, tag="xT")
        w_sb = persist.tile([128, 6, 1152], BF, tag="w")
        qT_sb = persist.tile([128, 3, N], BF, tag="qT")
        kT_sb = persist.tile([128, 3, N], BF, tag="kT")
        v_ones = persist.tile([128, NKT, HG, 65], BF, tag="vo")
        m01_sb = persist.tile([128, NKT, N], BF, tag="m01")
        attn_sb = persist.tile([128, 3, N], BF, tag="attn")
        wp_sb = persist.tile([128, 3, C], BF, tag="wp")
        ones_sb = persist.tile([128, 64], F32, tag="ones")
        nc.vector.memset(ones_sb[:, :], 1.0)

        for cc in range(6):
            nc.sync.dma_start(out=xT_sb[:, cc, :], in_=xT_d[cc])
            nc.sync.dma_start(out=w_sb[:, cc, :], in_=w_d[cc])
        for kt in range(4):
            nc.sync.dma_start(out=m01_sb[:, kt, :], in_=m_d[kt])
        for j in range(3):
            nc.sync.dma_start(out=wp_sb[:, j, :], in_=wp_d[j])
        for kt in range(4, NKT):
            nc.sync.dma_start(out=m01_sb[:, kt, :], in_=m_d[kt])

        # ones column of v_ones (col 64 of each [*, kt, h, :] block)
        nc.vector.memset(v_ones[:, :, :, 64:65], 1.0)

        # ---- phase 1 helpers (emitted interleaved with attention) ----
        def emit_qkv_cp(pool, cp):
            dest = qT_sb if cp < 3 else kT_sb
            j = cp % 3
            for nch in range(4):
                ps = pool.tile([128, 2 * WQ], F32, tag="s")
                for cc in range(6):
                    nc.tensor.matmul(
                        ps[:, 0:512],
                        w_sb[:, cc, cp * 128:(cp + 1) * 128],
                        xT_sb[:, cc, nch * 512:(nch + 1) * 512],
                        start=(cc == 0),
                        stop=(cc == 5),
                    )
                nc.vector.tensor_copy(
                    out=dest[:, j, nch * 512:(nch + 1) * 512], in_=ps[:, 0:512])

        def emit_v(pool):
            for nt in range(NKT):
                ps = pool.tile([128, 2 * WQ], F32, tag="s")
                for cc in range(6):
                    nc.tensor.matmul(
                        ps[:, 0:384],
                        xT_sb[:, cc, nt * 128:(nt + 1) * 128],
                        w_sb[:, cc, 768:1152],
                        start=(cc == 0),
                        stop=(cc == 5),
                    )
                nc.vector.tensor_copy(
                    out=v_ones[:, nt, :, 0:64],
                    in_=ps[:, 0:384].rearrange("p (h d) -> p h d", h=HG),
                )

        if dbg is not None:
            nc.sync.dma_start(out=dbg["d_qT"].ap(), in_=qT_sb[:, :, :])
            nc.sync.dma_start(out=dbg["d_kT"].ap(), in_=kT_sb[:, :, :])
            nc.sync.dma_start(out=dbg["d_vo"].ap(), in_=v_ones[:, :, :, :])

        # ---- phase 2: attention (qc-major, head-pair batched exp) ----
        dram = ctx.enter_context(tc.tile_pool(name="dscratch", bufs=1, space="DRAM"))
        rscratch = dram.tile([2 * NQC * HG, WQ], F32, tag="rs")
        with tc.tile_pool(name="ps_s", bufs=2, space="PSUM") as ps_s, \
             tc.tile_pool(name="ps_o", bufs=3, space="PSUM") as ps_o, \
             tc.tile_pool(name="ps_y", bufs=1, space="PSUM") as ps_y:
            emit_qkv_cp(ps_s, 0)
            emit_qkv_cp(ps_s, 3)
            emit_v(ps_s)
            for qc in range(NQC):
                q0 = qc * WQ
                for jp in range(3):          # head pair = (2*jp, 2*jp+1)
                    if qc == 0 and jp > 0:
                        emit_qkv_cp(ps_s, jp)
                        emit_qkv_cp(ps_s, jp + 3)
                    psum_oe = ps_o.tile([65, WQ], F32, tag="o")
                    psum_oo = ps_o.tile([65, WQ], F32, tag="o")
                    for kt in range(NKT):
                        psum_s = ps_s.tile([128, 2 * WQ], F32, tag="s")
                        k0 = kt * 128
                        nc.tensor.matmul(
                            psum_s[:, 0:WQ],
                            kT_sb[0:64, jp, k0:k0 + 128],
                            qT_sb[0:64, jp, q0:q0 + WQ],
                            start=True,
                            stop=True,
                        )
                        nc.tensor.matmul(
                            psum_s[:, WQ:2 * WQ],
                            kT_sb[64:128, jp, k0:k0 + 128],
                            qT_sb[64:128, jp, q0:q0 + WQ],
                            start=True,
                            stop=True,
                        )
                        p_raw = work.tile([128, 2 * WQ], BF, tag="praw")
                        nc.scalar.activation(
                            out=p_raw[:, :],
                            in_=psum_s[:, :],
                            func=(mybir.ActivationFunctionType.Copy
                                  if "noexp" in ABLATE else
                                  mybir.ActivationFunctionType.Exp),
                            scale=SCALE,
                        )
                        pT = work.tile([128, 2 * WQ], BF, tag="pt")
                        if "nomask" in ABLATE:
                            pT = p_raw
                        else:
                            msl = m01_sb[:, kt, q0:q0 + WQ]
                            msrc = bass.AP(
                                tensor=msl.tensor,
                                offset=msl.offset,
                                ap=[list(msl.ap[0]), [0, 2], [1, WQ]],
                            )
                            nc.vector.tensor_mul(
                                pT.rearrange("p (a b) -> p a b", a=2),
                                p_raw.rearrange("p (a b) -> p a b", a=2),
                                msrc,
                            )
                        if dbg is not None and jp == 0 and qc == 0 and kt == 0:
                            nc.sync.dma_start(out=dbg["d_praw"].ap(), in_=p_raw[:, 0:WQ])
                            nc.sync.dma_start(out=dbg["d_pt"].ap(), in_=pT[:, 0:WQ])
                        nc.tensor.matmul(
                            psum_oe[:, :],
                            v_ones[:, kt, 2 * jp, :],
                            pT[:, 0:WQ],
                            start=(kt == 0),
                            stop=(kt == NKT - 1),
                        )
                        nc.tensor.matmul(
                            psum_oo[:, :],
                            v_ones[:, kt, 2 * jp + 1, :],
                            pT[:, WQ:2 * WQ],
                            start=(kt == 0),
                            stop=(kt == NKT - 1),
                        )
                    for half, psum_o in ((0, psum_oe), (1, psum_oo)):
                        off = 64 * half
                        # unnormalized data straight into attn_sb (bf16)
                        nc.vector.tensor_copy(
                            out=attn_sb[off:off + 64, jp, q0:q0 + WQ],
                            in_=psum_o[0:64, :],
                        )
                        if "nonorm" in ABLATE:
                            continue
                        den = work2.tile([1, WQ], F32, tag="dn")
                        nc.vector.tensor_copy(out=den[:, :], in_=psum_o[64:65, :])
                        row = qc * HG + jp * 2 + half
                        nc.sync.dma_start(
                            out=rscratch[row:row + 1, :], in_=den[:, :]
                        )
                if "nonorm" not in ABLATE:
                    r0 = qc * HG
                    coll = work2.tile([HG, WQ], F32, tag="coll")
                    nc.sync.dma_start(
                        out=coll[:, :], in_=rscratch[r0:r0 + HG, :]
                    )
                    nc.vector.reciprocal(out=coll[:, :], in_=coll[:, :])
                    r1 = NQC * HG + r0
                    nc.sync.dma_start(
                        out=rscratch[r1:r1 + HG, :], in_=coll[:, :]
                    )
                    bc_all = work2.tile([128, 3, WQ], F32, tag="bca")
                    rows = rscratch[r1:r1 + HG, :]
                    for half in range(2):
                        bsrc = bass.AP(
                            tensor=rows.tensor,
                            offset=rows.offset + half * WQ,
                            ap=[[0, 64], [2 * WQ, 3], [1, WQ]],
                        )
                        nc.sync.dma_start(
                            out=bc_all[64 * half:64 * half + 64, :, :],
                            in_=bsrc,
                        )
                    for jp in range(3):
                        nc.vector.tensor_mul(
                            attn_sb[:, jp, q0:q0 + WQ],
                            attn_sb[:, jp, q0:q0 + WQ],
                            bc_all[:, jp, :],
                        )
                # proj for the n-tiles covered by this q-chunk
                for nt in range(qc * (WQ // 128), (qc + 1) * (WQ // 128)):
                    y_sb = work2.tile([128, C], F32, tag="ysb")
                    for colh in range(2):
                        c0 = colh * 384
                        psum_y = ps_y.tile([128, 384], F32, tag="y")
                        for j in range(3):
                            nc.tensor.matmul(
                                psum_y[:, :],
                                attn_sb[:, j, nt * 128:(nt + 1) * 128],
                                wp_sb[:, j, c0:c0 + 384],
                                start=(j == 0),
                                stop=(j == 2),
                            )
                        nc.vector.tensor_copy(
                            out=y_sb[:, c0:c0 + 384], in_=psum_y[:, :]
                        )
                    nc.sync.dma_start(
                        out=y_d[nt * 128:(nt + 1) * 128, :], in_=y_sb[:, :]
                    )

        if dbg is not None:
            nc.sync.dma_start(out=dbg["d_attn"].ap(), in_=attn_sb[:, :, :])


def _prep_inputs(x, mask, w_qkv, w_proj):
    """Build the 8 per-core input maps."""
    x = np.asarray(x, dtype=np.float32)
    mask = np.asarray(mask)
    w_qkv = np.asarray(w_qkv, dtype=np.float32)
    w_proj = np.asarray(w_proj, dtype=np.float32)

    m01T = np.ascontiguousarray((~mask).T.astype(np.float32)).astype(BF16)
    m01T = m01T.reshape(NKT, 128, N)

    w3 = w_qkv.reshape(C, 3, H, D)
    wp3 = w_proj.reshape(H, D, C)

    in_maps = []
    for core in range(NCORES):
        b, g = core // 2, core % 2
        hs = slice(g * HG, (g + 1) * HG)
        xT = np.ascontiguousarray(x[b].T).astype(BF16).reshape(6, 128, N)
        wq = w3[:, 0, hs, :].reshape(C, HG * D)
        wk = w3[:, 1, hs, :].reshape(C, HG * D)
        wv = w3[:, 2, hs, :].reshape(C, HG * D)
        wg = np.concatenate([wq, wk, wv], axis=1).astype(BF16)
        wg = np.ascontiguousarray(wg).reshape(6, 128, 1152)
        wp = np.ascontiguousarray(wp3[hs].reshape(3, 128, C)).astype(BF16)
        in_maps.append({"xT": xT, "wqkv": wg, "maskT": m01T, "wp": wp})
    return in_maps


_NC_CACHE = {}


def run_cores(in_maps, trace=False, **kw):
    if "nc" not in _NC_CACHE:
        _NC_CACHE["nc"] = _build_nc()
    nc = _NC_CACHE["nc"]
    return run_bass_kernel_spmd(
        nc, in_maps, core_ids=list(range(NCORES)), trace=trace, **kw
    )


def kernel(x, mask, w_qkv, w_proj, b_proj):
    in_maps = _prep_inputs(x, mask, w_qkv, w_proj)
    res = run_cores(in_maps)
    b_proj = np.asarray(b_proj, dtype=np.float32)
    out = np.empty((B, N, C), dtype=np.float32)
    for b in range(B):
        out[b] = (
            res.results[2 * b]["y"] + res.results[2 * b + 1]["y"] + b_proj
        )
    return out

